# revision 30
# baseline (speedup 1.0000x reference)
"""Trainium2 Bass kernel for nn_ACCriticNSMsg (per-agent MLP critic with
message passing).

Math (per batch b, time t, agent a):
    inp   = concat(obs[b,t,a], messages[b,t-1,j != a])          # 128 + 112
    h     = relu(inp @ W1[a] + b1[a])                            # 240 -> 64
    q     = h @ W2[a] + b2[a]                                    # 64 -> 1

Reformulation: the "other agents' previous messages" gather is replaced by
the FULL 128-dim message vector of all 8 agents at t-1 multiplied against an
expanded weight matrix W1m[a] whose rows for agent a's own message slice are
zero.  Layer 1 becomes two K=128 matmuls accumulated in PSUM:

    h = relu(obs[b,t,a] @ W1o[a] + full_msg[b,t-1] @ W1m[a] + b1[a])

Since the msg rhs (msgT) is shared by all agents, the two agents of a pair
share ONE msg matmul with a [128, 128] stationary [W1m[a0] | W1m[a1]].

Distribution: pure data-parallel over the batch axis (bs=64 -> 8 cores x 8),
weights replicated; no collectives.

v2 changes over the first working kernel (35.4us):
  - PE p-state warmup: TRN2's PE clock ramps 0.65 -> 1.2 -> 2.4 GHz with
    sustained busy time (full speed only after ~3us of gapless execution).
    Dummy matmuls on scratch SBUF run during the DMA lead-in so real
    matmuls start at full clock.
  - Deep software pipelining: each pair's msg matmul (which only needs the
    weight blob + msgT) is emitted 4 pair-slots ahead of its obs matmuls,
    so the PE always has runnable work even while obs tiles are in flight.
  - Output DMA moves only the 8 meaningful partition rows (strided AP),
    32KB instead of 524KB.
  - DMA: 8KB/row descriptors, split across the SP and ACT HWDGE queues so
    the 16 DMA engines stay fed; ~5MB total at ~360GB/s.
"""

import numpy as np
import ml_dtypes

import concourse.bass as bass
import concourse.bacc as bacc
import concourse.tile as tile
from concourse import mybir
from concourse.bass_utils import run_bass_kernel_spmd
from concourse.bass_interp import get_hw_module

BF16 = mybir.dt.bfloat16
F32 = mybir.dt.float32
NPBF16 = ml_dtypes.bfloat16

# Problem shapes (hardcoded per spec)
BS, T, N, MSG_LEN, OBS_DIM, HID = 64, 256, 8, 16, 128, 64
N_CORES = 8
BS_LOC = BS // N_CORES          # 8 batches per core
TOK = BS_LOC * T                # 2048 (b, t) tokens per core (per agent)
TOK_ALL = N * TOK               # 16384 obs tokens per core
TT = 512                        # token tile (one PSUM bank of f32)
N_TT = TOK // TT                # 4 token tiles per agent
MSG_D = N * MSG_LEN             # 128 = full message vector
NP = N // 2                     # agent pairs
NK = N_TT * NP                  # 16 (tile, pair) work items
LOOKAHEAD = 4                   # msg matmul emitted this many pairs early
N_DUMMY = 6                     # PE warmup matmuls (512 cols each)

# blob column layout: [w1m(512) | msg(4*512) | w1o(512) | w2(128) | b1(4)]
CB_W1M = N * HID                # 512
CB_MSG = CB_W1M + N_TT * TT     # 2560
CB_W1O = CB_MSG + N * HID       # 3072
CB_W2 = CB_W1O + NP * 32        # 3200
CB = CB_W2 + NP                 # 3204
BH = CB_W1M + TT                # head 1: w1m + msg0 only (SP, first)


def emit_l2(nc, q_ps, w2_sb, p, ht):
    """Layer-2 matmul for pair p: M=32 (cols 2..31 zero) so the 4 pairs
    jointly initialize every partition of the shared q bank."""
    nc.tensor.matmul(q_ps[32 * p:32 * (p + 1), :],
                     lhsT=w2_sb[:, p, :], rhs=ht[:],
                     start=True, stop=True,
                     tile_position=(0, 32 * p))


def _build():
    """Build the SPMD single-core graph (identical on all 8 cores)."""
    nc = bacc.Bacc("TRN2", target_bir_lowering=False, debug=False,
                   enable_asserts=False, num_devices=N_CORES)

    # ---- DRAM parameters (host pre-transposed/permuted, bf16) ----
    obs_d = nc.dram_tensor("obsT", [128, TOK_ALL], BF16, kind="ExternalInput").ap()
    cb_d = nc.dram_tensor("cblob16", [128, CB], BF16, kind="ExternalInput").ap()
    # q output: row (i, p) = agent 2p+i; cols (l, j) as in v1
    out_d = nc.dram_tensor("out", [2, NP, TOK], BF16, kind="ExternalOutput").ap()

    with tile.TileContext(nc) as tc:
        with tc.tile_pool(name="consts", bufs=1) as consts, \
             tc.tile_pool(name="acts", bufs=1) as acts, \
             tc.tile_pool(name="ht", bufs=6) as ht_pool, \
             tc.tile_pool(name="qsb", bufs=1) as qsb_pool, \
             tc.tile_pool(name="psl1", bufs=5, space="PSUM") as psl1, \
             tc.tile_pool(name="psl2", bufs=1, space="PSUM") as psl2:

            cb_sb = consts.tile([128, CB], BF16, tag="cb")
            obsT = acts.tile([128, N_TT, N, TT], BF16, tag="obsT")
            scratch = acts.tile([128, 640], BF16, tag="scratch")
            obs_r = obs_d.rearrange("k (t x) -> k t x", t=N_TT)
            obs_r0 = obs_d.rearrange("k (t g x) -> k t g x", t=N_TT, g=2)

            # ---- PE warmup: memset scratch, then dummy matmuls so the PE
            # p-state ramps to full clock during the DMA lead-in ----
            nc.gpsimd.memset(scratch[:], 0)

            # ---- input DMAs.  SP ring: minimal blob head (w1m+msg0) then
            # obs quarter-tiles for t0 and halves for t1-3 in consumption
            # order.  ACT ring: rest of the blob (msg1-3, w1o, w2, b1). ----
            nc.sync.dma_start(out=cb_sb[:, 0:BH], in_=cb_d[:, 0:BH])
            nc.scalar.dma_start(out=cb_sb[:, BH:], in_=cb_d[:, BH:])
            for t in range(0, N_TT):
                nc.sync.dma_start(out=obsT[:, t, 0:4], in_=obs_r0[:, t, 0])
                nc.sync.dma_start(out=obsT[:, t, 4:8], in_=obs_r0[:, t, 1])

            # blob views
            w1m_sb = cb_sb[:, 0:CB_W1M].rearrange(
                "k (p h) -> k p h", p=NP)          # [128, 4, 128] pair-major
            msg_sb = cb_sb[:, CB_W1M:CB_MSG].rearrange(
                "k (t x) -> k t x", t=N_TT)        # [128, 4, 512]
            w1o_sb = cb_sb[:, CB_MSG:CB_W1O].rearrange(
                "k (a h) -> k a h", a=N)
            w2_sb = cb_sb[:, CB_W1O:CB_W2].rearrange(
                "k (p m) -> k p m", p=NP)          # [128, 4, 32]
            b1_bf = cb_sb[:, CB_W2:CB]
            b1_sb = consts.tile([128, NP], F32, tag="b1f32")
            nc.vector.tensor_copy(out=b1_sb[:], in_=b1_bf)

            # q staging [128, 2048] bf16 (only rows 32p+{0,1} meaningful)
            q_sb = qsb_pool.tile([128, TOK], BF16, tag="q_sb")

            dummy_ct = [0]

            def emit_dummy(cols, rhs=None):
                """Keep-warm matmul: holds the PE p-state at full clock
                through DMA-arrival jitter.  An rhs from live data pins the
                dummy in schedule order (the tile scheduler hoists
                dependency-free work to the front)."""
                d = dummy_ct[0]
                dummy_ct[0] += 1
                dps = psl2.tile([128, TT], F32, tag="warm", bufs=2,
                                name=f"warm{d}")
                if rhs is None:
                    rhs = scratch[:, 128:128 + cols]
                nc.tensor.matmul(dps[:, 0:cols], lhsT=scratch[:, 0:128],
                                 rhs=rhs,
                                 start=True, stop=True,
                                 skip_group_check=True)

            for d in range(N_DUMMY):
                emit_dummy(TT)

            def msg_mm(k):
                """L1 msg matmul for work item k = t*NP + p: opens the PSUM
                bank (start=True, M=128 covers all partitions)."""
                t, p = divmod(k, NP)
                ps = psl1.tile([128, TT], F32, tag="l1", name=f"l1_{k}")
                msg_rhs = msg_sb[:, t, :]
                nc.tensor.matmul(ps[:, :], lhsT=w1m_sb[:, p, :],
                                 rhs=msg_rhs,
                                 start=True, stop=False,
                                 skip_group_check=True)
                return ps

            # Software pipeline: L2s are emitted two at a time (pairs p0+p1 /
            # p2+p3 occupy disjoint 32-col PE quadrants and overlap); q copy
            # (column-split across ACT/DVE) + output DMAs fire per tile.
            q_tiles = {}
            pend = []

            def pop_pend2():
                for _ in range(2):
                    t_, p_, ht_ = pend.pop(0)
                    emit_l2(nc, q_tiles[t_], w2_sb, p_, ht_)
                if p_ == NP - 1:
                    sl_ = bass.ts(t_, TT)
                    h_ = TT // 2
                    nc.scalar.activation(
                        out=q_sb[:, t_ * TT:t_ * TT + h_],
                        in_=q_tiles[t_][:, 0:h_],
                        func=mybir.ActivationFunctionType.Copy)
                    nc.vector.tensor_copy(
                        out=q_sb[:, t_ * TT + h_:(t_ + 1) * TT],
                        in_=q_tiles[t_][:, h_:TT])
                    # strided-partition views: rows {32p} and {32p+1};
                    # issue from both queues so the two configs overlap
                    nc.scalar.dma_start(out=out_d[0, :, sl_],
                                        in_=q_sb[0:128:32, sl_])
                    nc.scalar.dma_start(out=out_d[1, :, sl_],
                                        in_=q_sb[1:128:32, sl_])

            mm_heads = [msg_mm(k) for k in range(LOOKAHEAD)]
            l1_ps = {k: mm_heads[k] for k in range(LOOKAHEAD)}

            for k in range(NK):
                t, p = divmod(k, NP)
                a0, a1 = 2 * p, 2 * p + 1
                if t not in q_tiles:
                    q_tiles[t] = psl2.tile([128, TT], F32, tag="q",
                                           bufs=1, name=f"q_ps{t}")
                # L2 pops lead each iteration: if the obs DMA is late the PE
                # still has runnable (DMA-independent) work in front of it.
                if len(pend) >= 2:
                    pop_pend2()
                ps = l1_ps.pop(k)
                nc.tensor.matmul(ps[0:HID, :], lhsT=w1o_sb[:, a0, :],
                                 rhs=obsT[:, t, a0, :],
                                 start=False, stop=False,
                                 skip_group_check=True)
                nc.tensor.matmul(ps[HID:128, :], lhsT=w1o_sb[:, a1, :],
                                 rhs=obsT[:, t, a1, :],
                                 start=False, stop=True,
                                 skip_group_check=True)
                # relu+bias, column-split across ACT and DVE concurrently
                ht = ht_pool.tile([128, TT], BF16, tag="ht", name=f"ht_{k}")
                h = TT // 2
                nc.scalar.activation(out=ht[:, 0:h], in_=ps[:, 0:h],
                                     func=mybir.ActivationFunctionType.Relu,
                                     bias=b1_sb[:, p:p + 1], scale=1.0)
                nc.vector.tensor_scalar(out=ht[:, h:TT], in0=ps[:, h:TT],
                                        scalar1=b1_sb[:, p:p + 1],
                                        scalar2=0.0,
                                        op0=mybir.AluOpType.add,
                                        op1=mybir.AluOpType.max)
                pend.append((t, p, ht))
                if k + LOOKAHEAD < NK:
                    l1_ps[k + LOOKAHEAD] = msg_mm(k + LOOKAHEAD)
            while pend:
                pop_pend2()

    nc.compile()
    nc.m = get_hw_module(nc.m)
    return nc


_NC_CACHE = None


def _get_nc():
    global _NC_CACHE
    if _NC_CACHE is None:
        _NC_CACHE = _build()
    return _NC_CACHE


def _prep_inputs(obs, messages, W1, b1, W2, b2):
    """Host-side shard + repack + transpose. Returns in_maps for 8 cores."""
    obs = np.asarray(obs, dtype=np.float32)
    messages = np.asarray(messages, dtype=np.float32)
    W1 = np.asarray(W1, dtype=np.float32)
    b1 = np.asarray(b1, dtype=np.float32)
    W2 = np.asarray(W2, dtype=np.float32)

    # expanded message weights (own-agent slice zeroed), matching reference's
    # [prev agents, next agents] concat order
    W1o = W1[:, :OBS_DIM, :]                         # [8, 128, 64]
    W1m = np.zeros((N, MSG_D, HID), np.float32)
    for a in range(N):
        k = 0
        for j in range(N):
            if j == a:
                continue
            W1m[a, j * MSG_LEN:(j + 1) * MSG_LEN] = \
                W1[a, OBS_DIM + k * MSG_LEN: OBS_DIM + (k + 1) * MSG_LEN]
            k += 1

    w1o_k = W1o.transpose(1, 0, 2).reshape(128, N * HID)
    w1m_k = W1m.transpose(1, 0, 2).reshape(128, N * HID)
    w2p = np.zeros((128, NP, 32), np.float32)
    for p in range(NP):
        w2p[0:HID, p, 0] = W2[2 * p, :, 0]
        w2p[HID:128, p, 1] = W2[2 * p + 1, :, 0]
    w2p = w2p.reshape(128, NP * 32)

    b1p = np.zeros((128, NP), np.float32)
    for p in range(NP):
        b1p[0:HID, p] = b1[2 * p]
        b1p[HID:128, p] = b1[2 * p + 1]
    b1p16 = b1p.astype(NPBF16)

    # shifted full message vector [bs, T, 128]
    msgf = messages.reshape(BS, T, MSG_D)
    msgs_shift = np.zeros_like(msgf)
    msgs_shift[:, 1:] = msgf[:, :-1]

    w1m16 = w1m_k.astype(NPBF16)
    wtail16 = np.concatenate([w1o_k, w2p], axis=1).astype(NPBF16)

    in_maps = []
    for c in range(N_CORES):
        bsl = slice(c * BS_LOC, (c + 1) * BS_LOC)
        # token order per agent: (l=t%16, j=b*16 + t//16); obs columns
        # grouped (toktile, agent, 512)
        ob = obs[bsl].reshape(BS_LOC, 16, 16, N, OBS_DIM)
        o = ob.transpose(4, 3, 2, 0, 1).reshape(128, N, N_TT, TT)
        o = np.ascontiguousarray(o.transpose(0, 2, 1, 3)).reshape(
            128, TOK_ALL).astype(NPBF16)
        mb = msgs_shift[bsl].reshape(BS_LOC, 16, 16, MSG_D)
        m = np.ascontiguousarray(mb.transpose(3, 2, 0, 1)).reshape(
            128, TOK).astype(NPBF16)
        cblob16 = np.concatenate(
            [w1m16, m, wtail16, b1p16], axis=1)
        in_maps.append({
            "obsT": o, "cblob16": cblob16,
        })
    return in_maps


def _install_profile_hook():
    """The boot environment lacks antenv.axon_hooks; install the NTFF hook ourselves."""
    import sys as _sys
    import types as _types
    try:
        from antenv.axon_hooks import get_axon_ntff_profile_hook  # noqa: F401
        return
    except ImportError:
        pass
    try:
        import antenv
        from trn_agent_boot.trn_boot import _ntff_profile_via_ctypes
        hook = _ntff_profile_via_ctypes("/opt/axon/libaxon_pjrt.so")
        mod = _types.ModuleType("antenv.axon_hooks")
        mod._hook = hook
        mod.get_axon_ntff_profile_hook = lambda: mod._hook

        def _set(h):
            mod._hook = h

        mod.set_axon_ntff_profile_hook = _set
        _sys.modules["antenv.axon_hooks"] = mod
        antenv.axon_hooks = mod
    except Exception as e:  # profiling is best-effort
        print(f"profile hook install failed: {e}")


def run(obs, messages, W1, b1, W2, b2, trace=False):
    if trace:
        _install_profile_hook()
    nc = _get_nc()
    in_maps = _prep_inputs(obs, messages, W1, b1, W2, b2)
    res = run_bass_kernel_spmd(nc, in_maps, core_ids=list(range(N_CORES)),
                               trace=trace)
    b2 = np.asarray(b2, dtype=np.float32)
    outs = []
    for c in range(N_CORES):
        o = np.asarray(res.results[c]["out"]).astype(np.float32)  # [2, 4, 2048]
        # row (i, p) = agent 2p+i; cols (l, j) with j = b*16 + t_hi,
        # t = t_hi*16 + l
        o = o.transpose(1, 0, 2)                       # [p, i, tok] -> agent order
        qa = o.reshape(N, 16, BS_LOC, 16)              # [a, l, b, t_hi]
        q = qa.transpose(2, 3, 1, 0)                   # [b, t_hi, l, a]
        q = q.reshape(BS_LOC, T, N, 1) + b2[None, None, :, :]
        outs.append(q)
    full = np.concatenate(outs, axis=0).astype(np.float32)
    return full, res


def kernel(obs, messages, W1, b1, W2, b2):
    out, _ = run(obs, messages, W1, b1, W2, b2, trace=False)
    return out


# revision 31
# speedup vs baseline: 1.0923x; 1.0923x over previous
"""Trainium2 Bass kernel for nn_ACCriticNSMsg (per-agent MLP critic with
message passing).

Math (per batch b, time t, agent a):
    inp   = concat(obs[b,t,a], messages[b,t-1,j != a])          # 128 + 112
    h     = relu(inp @ W1[a] + b1[a])                            # 240 -> 64
    q     = h @ W2[a] + b2[a]                                    # 64 -> 1

Reformulation: the "other agents' previous messages" gather is replaced by
the FULL 128-dim message vector of all 8 agents at t-1 multiplied against an
expanded weight matrix W1m[a] whose rows for agent a's own message slice are
zero.  Layer 1 becomes two K=128 matmuls accumulated in PSUM:

    h = relu(obs[b,t,a] @ W1o[a] + full_msg[b,t-1] @ W1m[a] + b1[a])

Since the msg rhs (msgT) is shared by all agents, the two agents of a pair
share ONE msg matmul with a [128, 128] stationary [W1m[a0] | W1m[a1]].

Distribution: pure data-parallel over the batch axis (bs=64 -> 8 cores x 8),
weights replicated; no collectives.

v2 changes over the first working kernel (35.4us):
  - PE p-state warmup: TRN2's PE clock ramps 0.65 -> 1.2 -> 2.4 GHz with
    sustained busy time (full speed only after ~3us of gapless execution).
    Dummy matmuls on scratch SBUF run during the DMA lead-in so real
    matmuls start at full clock.
  - Deep software pipelining: each pair's msg matmul (which only needs the
    weight blob + msgT) is emitted 4 pair-slots ahead of its obs matmuls,
    so the PE always has runnable work even while obs tiles are in flight.
  - Output DMA moves only the 8 meaningful partition rows (strided AP),
    32KB instead of 524KB.
  - DMA: 8KB/row descriptors, split across the SP and ACT HWDGE queues so
    the 16 DMA engines stay fed; ~5MB total at ~360GB/s.
"""

import numpy as np
import ml_dtypes

import concourse.bass as bass
import concourse.bacc as bacc
import concourse.tile as tile
from concourse import mybir
from concourse.bass_utils import run_bass_kernel_spmd
from concourse.bass_interp import get_hw_module

BF16 = mybir.dt.bfloat16
F32 = mybir.dt.float32
NPBF16 = ml_dtypes.bfloat16

# Problem shapes (hardcoded per spec)
BS, T, N, MSG_LEN, OBS_DIM, HID = 64, 256, 8, 16, 128, 64
N_CORES = 8
BS_LOC = BS // N_CORES          # 8 batches per core
TOK = BS_LOC * T                # 2048 (b, t) tokens per core (per agent)
TOK_ALL = N * TOK               # 16384 obs tokens per core
TT = 512                        # token tile (one PSUM bank of f32)
N_TT = TOK // TT                # 4 token tiles per agent
MSG_D = N * MSG_LEN             # 128 = full message vector
NP = N // 2                     # agent pairs
NK = N_TT * NP                  # 16 (tile, pair) work items
LOOKAHEAD = 4                   # msg matmul emitted this many pairs early
N_DUMMY = 9                     # PE warmup matmuls (512 cols each)

# blob column layout: [w1m(512) | msg(4*512) | w1o(512) | w2(128) | b1(4)]
CB_W1M = N * HID                # 512
CB_MSG = CB_W1M + N_TT * TT     # 2560
CB_W1O = CB_MSG + N * HID       # 3072
CB_W2 = CB_W1O + NP * 32        # 3200
CB = CB_W2 + NP                 # 3204
BH = CB_W1M + TT                # head 1: w1m + msg0 only (SP, first)


def emit_l2(nc, q_ps, w2_sb, p, ht):
    """Layer-2 matmul for pair p: M=32 (cols 2..31 zero) so the 4 pairs
    jointly initialize every partition of the shared q bank."""
    nc.tensor.matmul(q_ps[32 * p:32 * (p + 1), :],
                     lhsT=w2_sb[:, p, :], rhs=ht[:],
                     start=True, stop=True,
                     tile_position=(0, 32 * p))


def _build():
    """Build the SPMD single-core graph (identical on all 8 cores)."""
    nc = bacc.Bacc("TRN2", target_bir_lowering=False, debug=False,
                   enable_asserts=False, num_devices=N_CORES)

    # ---- DRAM parameters (host pre-transposed/permuted, bf16) ----
    obs_d = nc.dram_tensor("obsT", [128, TOK_ALL], BF16, kind="ExternalInput").ap()
    cb_d = nc.dram_tensor("cblob16", [128, CB], BF16, kind="ExternalInput").ap()
    # q output: row (i, p) = agent 2p+i; cols (l, j) as in v1
    out_d = nc.dram_tensor("out", [2, NP, TOK], BF16, kind="ExternalOutput").ap()

    with tile.TileContext(nc) as tc:
        with tc.tile_pool(name="consts", bufs=1) as consts, \
             tc.tile_pool(name="acts", bufs=1) as acts, \
             tc.tile_pool(name="ht", bufs=6) as ht_pool, \
             tc.tile_pool(name="qsb", bufs=1) as qsb_pool, \
             tc.tile_pool(name="psl1", bufs=5, space="PSUM") as psl1, \
             tc.tile_pool(name="psl2", bufs=1, space="PSUM") as psl2:

            cb_sb = consts.tile([128, CB], BF16, tag="cb")
            obsT = acts.tile([128, N_TT, N, TT], BF16, tag="obsT")
            scratch = acts.tile([128, 640], BF16, tag="scratch")
            obs_r = obs_d.rearrange("k (t x) -> k t x", t=N_TT)
            obs_r0 = obs_d.rearrange("k (t g x) -> k t g x", t=N_TT, g=2)

            # ---- PE warmup: memset scratch, then dummy matmuls so the PE
            # p-state ramps to full clock during the DMA lead-in ----
            nc.gpsimd.memset(scratch[:], 0)

            # ---- input DMAs.  SP ring: minimal blob head (w1m+msg0) then
            # obs quarter-tiles for t0 and halves for t1-3 in consumption
            # order.  ACT ring: rest of the blob (msg1-3, w1o, w2, b1). ----
            nc.sync.dma_start(out=cb_sb[:, 0:BH], in_=cb_d[:, 0:BH])
            nc.scalar.dma_start(out=cb_sb[:, BH:], in_=cb_d[:, BH:])
            for t in range(0, N_TT):
                nc.sync.dma_start(out=obsT[:, t, 0:4], in_=obs_r0[:, t, 0])
                nc.sync.dma_start(out=obsT[:, t, 4:8], in_=obs_r0[:, t, 1])

            # blob views
            w1m_sb = cb_sb[:, 0:CB_W1M].rearrange(
                "k (p h) -> k p h", p=NP)          # [128, 4, 128] pair-major
            msg_sb = cb_sb[:, CB_W1M:CB_MSG].rearrange(
                "k (t x) -> k t x", t=N_TT)        # [128, 4, 512]
            w1o_sb = cb_sb[:, CB_MSG:CB_W1O].rearrange(
                "k (a h) -> k a h", a=N)
            w2_sb = cb_sb[:, CB_W1O:CB_W2].rearrange(
                "k (p m) -> k p m", p=NP)          # [128, 4, 32]
            b1_bf = cb_sb[:, CB_W2:CB]
            b1_sb = consts.tile([128, NP], F32, tag="b1f32")
            nc.vector.tensor_copy(out=b1_sb[:], in_=b1_bf)

            # q staging [128, 2048] bf16 (only rows 32p+{0,1} meaningful)
            q_sb = qsb_pool.tile([128, TOK], BF16, tag="q_sb")

            dummy_ct = [0]

            def emit_dummy(cols, rhs=None):
                """Keep-warm matmul: holds the PE p-state at full clock
                through DMA-arrival jitter.  An rhs from live data pins the
                dummy in schedule order (the tile scheduler hoists
                dependency-free work to the front)."""
                d = dummy_ct[0]
                dummy_ct[0] += 1
                dps = psl2.tile([128, TT], F32, tag="warm", bufs=2,
                                name=f"warm{d}")
                if rhs is None:
                    rhs = scratch[:, 128:128 + cols]
                nc.tensor.matmul(dps[:, 0:cols], lhsT=scratch[:, 0:128],
                                 rhs=rhs,
                                 start=True, stop=True,
                                 skip_group_check=True)

            for d in range(N_DUMMY):
                emit_dummy(TT)

            def msg_mm(k):
                """L1 msg matmul for work item k = t*NP + p: opens the PSUM
                bank (start=True, M=128 covers all partitions)."""
                t, p = divmod(k, NP)
                ps = psl1.tile([128, TT], F32, tag="l1", name=f"l1_{k}")
                msg_rhs = msg_sb[:, t, :]
                nc.tensor.matmul(ps[:, :], lhsT=w1m_sb[:, p, :],
                                 rhs=msg_rhs,
                                 start=True, stop=False,
                                 skip_group_check=True)
                return ps

            # Software pipeline: L2s are emitted two at a time (pairs p0+p1 /
            # p2+p3 occupy disjoint 32-col PE quadrants and overlap); q copy
            # (column-split across ACT/DVE) + output DMAs fire per tile.
            q_tiles = {}
            pend = []

            def pop_pend4():
                for _ in range(4):
                    t_, p_, ht_ = pend.pop(0)
                    emit_l2(nc, q_tiles[t_], w2_sb, p_, ht_)
                if p_ == NP - 1:
                    sl_ = bass.ts(t_, TT)
                    h_ = TT // 2
                    nc.scalar.activation(
                        out=q_sb[:, t_ * TT:t_ * TT + h_],
                        in_=q_tiles[t_][:, 0:h_],
                        func=mybir.ActivationFunctionType.Copy)
                    nc.vector.tensor_copy(
                        out=q_sb[:, t_ * TT + h_:(t_ + 1) * TT],
                        in_=q_tiles[t_][:, h_:TT])
                    # strided-partition views: rows {32p} and {32p+1};
                    # issue from both queues so the two configs overlap
                    nc.sync.dma_start(out=out_d[0, :, sl_],
                                      in_=q_sb[0:128:32, sl_])
                    nc.sync.dma_start(out=out_d[1, :, sl_],
                                      in_=q_sb[1:128:32, sl_])

            mm_heads = [msg_mm(k) for k in range(LOOKAHEAD)]
            l1_ps = {k: mm_heads[k] for k in range(LOOKAHEAD)}

            for k in range(NK):
                t, p = divmod(k, NP)
                a0, a1 = 2 * p, 2 * p + 1
                if t not in q_tiles:
                    q_tiles[t] = psl2.tile([128, TT], F32, tag="q",
                                           bufs=1, name=f"q_ps{t}")
                # L2 quad pops lead the iteration after a tile completes:
                # all four pairs' L2s overlap in disjoint 32-col quadrants.
                if len(pend) >= 4:
                    pop_pend4()
                ps = l1_ps.pop(k)
                nc.tensor.matmul(ps[0:HID, :], lhsT=w1o_sb[:, a0, :],
                                 rhs=obsT[:, t, a0, :],
                                 start=False, stop=False,
                                 skip_group_check=True)
                nc.tensor.matmul(ps[HID:128, :], lhsT=w1o_sb[:, a1, :],
                                 rhs=obsT[:, t, a1, :],
                                 start=False, stop=True,
                                 skip_group_check=True)
                # relu+bias, column-split across ACT and DVE concurrently
                ht = ht_pool.tile([128, TT], BF16, tag="ht", name=f"ht_{k}")
                h = TT // 2
                nc.scalar.activation(out=ht[:, 0:h], in_=ps[:, 0:h],
                                     func=mybir.ActivationFunctionType.Relu,
                                     bias=b1_sb[:, p:p + 1], scale=1.0)
                nc.vector.tensor_scalar(out=ht[:, h:TT], in0=ps[:, h:TT],
                                        scalar1=b1_sb[:, p:p + 1],
                                        scalar2=0.0,
                                        op0=mybir.AluOpType.add,
                                        op1=mybir.AluOpType.max)
                pend.append((t, p, ht))
                if k + LOOKAHEAD < NK:
                    l1_ps[k + LOOKAHEAD] = msg_mm(k + LOOKAHEAD)
            while pend:
                pop_pend4()

    nc.compile()
    nc.m = get_hw_module(nc.m)
    return nc


_NC_CACHE = None


def _get_nc():
    global _NC_CACHE
    if _NC_CACHE is None:
        _NC_CACHE = _build()
    return _NC_CACHE


def _prep_inputs(obs, messages, W1, b1, W2, b2):
    """Host-side shard + repack + transpose. Returns in_maps for 8 cores."""
    obs = np.asarray(obs, dtype=np.float32)
    messages = np.asarray(messages, dtype=np.float32)
    W1 = np.asarray(W1, dtype=np.float32)
    b1 = np.asarray(b1, dtype=np.float32)
    W2 = np.asarray(W2, dtype=np.float32)

    # expanded message weights (own-agent slice zeroed), matching reference's
    # [prev agents, next agents] concat order
    W1o = W1[:, :OBS_DIM, :]                         # [8, 128, 64]
    W1m = np.zeros((N, MSG_D, HID), np.float32)
    for a in range(N):
        k = 0
        for j in range(N):
            if j == a:
                continue
            W1m[a, j * MSG_LEN:(j + 1) * MSG_LEN] = \
                W1[a, OBS_DIM + k * MSG_LEN: OBS_DIM + (k + 1) * MSG_LEN]
            k += 1

    w1o_k = W1o.transpose(1, 0, 2).reshape(128, N * HID)
    w1m_k = W1m.transpose(1, 0, 2).reshape(128, N * HID)
    w2p = np.zeros((128, NP, 32), np.float32)
    for p in range(NP):
        w2p[0:HID, p, 0] = W2[2 * p, :, 0]
        w2p[HID:128, p, 1] = W2[2 * p + 1, :, 0]
    w2p = w2p.reshape(128, NP * 32)

    b1p = np.zeros((128, NP), np.float32)
    for p in range(NP):
        b1p[0:HID, p] = b1[2 * p]
        b1p[HID:128, p] = b1[2 * p + 1]
    b1p16 = b1p.astype(NPBF16)

    # shifted full message vector [bs, T, 128]
    msgf = messages.reshape(BS, T, MSG_D)
    msgs_shift = np.zeros_like(msgf)
    msgs_shift[:, 1:] = msgf[:, :-1]

    w1m16 = w1m_k.astype(NPBF16)
    wtail16 = np.concatenate([w1o_k, w2p], axis=1).astype(NPBF16)

    in_maps = []
    for c in range(N_CORES):
        bsl = slice(c * BS_LOC, (c + 1) * BS_LOC)
        # token order per agent: (l=t%16, j=b*16 + t//16); obs columns
        # grouped (toktile, agent, 512)
        ob = obs[bsl].reshape(BS_LOC, 16, 16, N, OBS_DIM)
        o = ob.transpose(4, 3, 2, 0, 1).reshape(128, N, N_TT, TT)
        o = np.ascontiguousarray(o.transpose(0, 2, 1, 3)).reshape(
            128, TOK_ALL).astype(NPBF16)
        mb = msgs_shift[bsl].reshape(BS_LOC, 16, 16, MSG_D)
        m = np.ascontiguousarray(mb.transpose(3, 2, 0, 1)).reshape(
            128, TOK).astype(NPBF16)
        cblob16 = np.concatenate(
            [w1m16, m, wtail16, b1p16], axis=1)
        in_maps.append({
            "obsT": o, "cblob16": cblob16,
        })
    return in_maps


def _install_profile_hook():
    """The boot environment lacks antenv.axon_hooks; install the NTFF hook ourselves."""
    import sys as _sys
    import types as _types
    try:
        from antenv.axon_hooks import get_axon_ntff_profile_hook  # noqa: F401
        return
    except ImportError:
        pass
    try:
        import antenv
        from trn_agent_boot.trn_boot import _ntff_profile_via_ctypes
        hook = _ntff_profile_via_ctypes("/opt/axon/libaxon_pjrt.so")
        mod = _types.ModuleType("antenv.axon_hooks")
        mod._hook = hook
        mod.get_axon_ntff_profile_hook = lambda: mod._hook

        def _set(h):
            mod._hook = h

        mod.set_axon_ntff_profile_hook = _set
        _sys.modules["antenv.axon_hooks"] = mod
        antenv.axon_hooks = mod
    except Exception as e:  # profiling is best-effort
        print(f"profile hook install failed: {e}")


def run(obs, messages, W1, b1, W2, b2, trace=False):
    if trace:
        _install_profile_hook()
    nc = _get_nc()
    in_maps = _prep_inputs(obs, messages, W1, b1, W2, b2)
    res = run_bass_kernel_spmd(nc, in_maps, core_ids=list(range(N_CORES)),
                               trace=trace)
    b2 = np.asarray(b2, dtype=np.float32)
    outs = []
    for c in range(N_CORES):
        o = np.asarray(res.results[c]["out"]).astype(np.float32)  # [2, 4, 2048]
        # row (i, p) = agent 2p+i; cols (l, j) with j = b*16 + t_hi,
        # t = t_hi*16 + l
        o = o.transpose(1, 0, 2)                       # [p, i, tok] -> agent order
        qa = o.reshape(N, 16, BS_LOC, 16)              # [a, l, b, t_hi]
        q = qa.transpose(2, 3, 1, 0)                   # [b, t_hi, l, a]
        q = q.reshape(BS_LOC, T, N, 1) + b2[None, None, :, :]
        outs.append(q)
    full = np.concatenate(outs, axis=0).astype(np.float32)
    return full, res


def kernel(obs, messages, W1, b1, W2, b2):
    out, _ = run(obs, messages, W1, b1, W2, b2, trace=False)
    return out
